# revision 1
# baseline (speedup 1.0000x reference)
"""ComplexCoordAttention kernel — data-parallel over batch across 8 NeuronCores.

Sharding: B=8 batch elements -> one per core (data-parallel, no collectives).
Each core computes its full [S, Nl, 3] output slice; results are gathered on
host into the full [B, S, Nl, 3] array.

Self-contained: hardcodes shapes B=8, S=64, P=64, Nl=128, Np=256.
"""

import numpy as np
import jax
import jax.numpy as jnp

EPS = 1e-6


def _safe_norm(x, axis, keepdims=True):
    sq = (x * x).sum(axis=axis, keepdims=keepdims)
    safe = jnp.where(sq > 0, sq, 1.0)
    return jnp.where(sq > 0, jnp.sqrt(safe), 0.0)


def _complex_coord_norm(lig, pro, lig_mask, pro_mask, set_weights):
    lig_len = _safe_norm(lig, axis=-1, keepdims=False) * lig_mask   # [B,S,Nl]
    pro_len = _safe_norm(pro, axis=-1, keepdims=False) * pro_mask   # [B,S,Np]
    total_len = lig_len.sum(-1) + pro_len.sum(-1)                   # [B,S]
    n_atoms = lig_mask.sum(-1) + pro_mask.sum(-1)                   # [B,S]
    mean_len = (total_len / n_atoms)[..., None, None]               # [B,S,1,1]
    w = set_weights[None, :, None, None]
    lig = lig * w / (mean_len + EPS) * lig_mask[..., None]
    pro = pro * w / (mean_len + EPS) * pro_mask[..., None]
    return lig, pro


def _forward(lig_coord_sets, messages, adj_matrix, lig_node_mask,
             pro_coord_sets, pro_node_mask,
             lig_proj_w, pro_proj_w, attn_proj_w, set_weights):
    # Shapes here carry a leading batch dim of 1 (per-device shard).
    B, S, Nl, _ = lig_coord_sets.shape
    lig, pro = _complex_coord_norm(lig_coord_sets, pro_coord_sets,
                                   lig_node_mask, pro_node_mask, set_weights)
    lig_p = jnp.einsum('bsnc,ps->bcnp', lig, lig_proj_w)
    pro_p = jnp.einsum('bsnc,ps->bcnp', pro, pro_proj_w)
    proj = jnp.concatenate([lig_p, pro_p], axis=2)                # [B,3,N,P]

    # Decomposed pairwise computation (avoids materializing [B,3,Nl,N,P]
    # all at once on host, but on-device jax fuses fine; keep it direct):
    vec = proj[:, :, :Nl, None, :] - proj[:, :, None, :, :]       # [B,3,Nl,N,P]
    lengths = _safe_norm(vec, axis=1)                             # [B,1,Nl,N,P]
    norm_dists = vec / (lengths + EPS)
    attn_mask = jnp.where(adj_matrix > 0, 0.0, -jnp.inf)          # [B,Nl,N]
    attn = jax.nn.softmax(messages + attn_mask[..., None], axis=2)
    upd = jnp.einsum('bcinp,binp->bcip', norm_dists, attn)        # [B,3,Nl,P]
    weights = jnp.sqrt((attn * attn).sum(axis=2))                 # [B,Nl,P]
    upd = upd * weights[:, None]
    return jnp.einsum('bcip,sp->bsic', upd, attn_proj_w)          # [B,S,Nl,3]


_pmapped = None


def _get_pmapped():
    global _pmapped
    if _pmapped is None:
        _pmapped = jax.pmap(
            _forward,
            in_axes=(0, 0, 0, 0, 0, 0, None, None, None, None),
        )
    return _pmapped


def kernel(**inputs) -> np.ndarray:
    lig = np.asarray(inputs["lig_coord_sets"], np.float32)    # [8,64,128,3]
    msg = np.asarray(inputs["messages"], np.float32)          # [8,128,384,64]
    adj = np.asarray(inputs["adj_matrix"], np.int32)          # [8,128,384]
    ligm = np.asarray(inputs["lig_node_mask"], np.float32)    # [8,64,128]
    pro = np.asarray(inputs["pro_coord_sets"], np.float32)    # [8,64,256,3]
    prom = np.asarray(inputs["pro_node_mask"], np.float32)    # [8,64,256]
    wl = np.asarray(inputs["lig_proj_w"], np.float32)         # [64,64]
    wp = np.asarray(inputs["pro_proj_w"], np.float32)         # [64,64]
    wa = np.asarray(inputs["attn_proj_w"], np.float32)        # [64,64]
    sw = np.asarray(inputs["set_weights"], np.float32)        # [64]

    B = lig.shape[0]
    n_dev = min(B, len(jax.devices()))
    assert B % n_dev == 0
    per = B // n_dev

    # Shard batch across devices: leading pmap axis = device, keep a
    # per-device batch dim so _forward's batched code runs unchanged.
    def shard(x):
        return x.reshape((n_dev, per) + x.shape[1:])

    out = _get_pmapped()(
        shard(lig), shard(msg), shard(adj), shard(ligm), shard(pro),
        shard(prom), wl, wp, wa, sw,
    )
    out = np.asarray(out)                       # [n_dev, per, S, Nl, 3]
    return out.reshape((B,) + out.shape[2:]).astype(np.float32)


if __name__ == "__main__":
    # Smoke test with random data of the right shapes.
    rng = np.random.default_rng(0)
    ins = {
        "lig_coord_sets": rng.standard_normal((8, 64, 128, 3), np.float32),
        "messages": rng.standard_normal((8, 128, 384, 64), np.float32),
        "adj_matrix": rng.integers(0, 2, (8, 128, 384)).astype(np.int32),
        "lig_node_mask": np.ones((8, 64, 128), np.float32),
        "pro_coord_sets": rng.standard_normal((8, 64, 256, 3), np.float32),
        "pro_node_mask": np.ones((8, 64, 256), np.float32),
        "lig_proj_w": rng.standard_normal((64, 64), np.float32) * 0.125,
        "pro_proj_w": rng.standard_normal((64, 64), np.float32) * 0.125,
        "attn_proj_w": rng.standard_normal((64, 64), np.float32) * 0.125,
        "set_weights": np.ones((64,), np.float32),
    }
    print(kernel(**ins).shape)
